# revision 9
# baseline (speedup 1.0000x reference)
"""Trainium2 Bass kernel for nn_Camada_33612414059004.

Computes, for x:[B,N,D,S], M:[N,N], w_syn:[N,D,S], b_dend:[N,D],
w_dend:[N,D], b_soma:[N]:

    xm    = einsum('bids,oi->bods', x, M)
    dend  = tanh(einsum('bnds,nds->bnd', xm, w_syn) + b_dend)
    soma  = einsum('bnd,nd->bn', dend, w_dend) + b_soma
    out   = sigmoid(soma)                                  # [B, N]

Sharding: data-parallel over batch across 8 NeuronCores (B=64 -> 8/core),
zero cross-core communication.  Per core the dominant work is the
connectivity matmul  M[o,i] @ x[i, (b,d,s)]  ([1024x1024]x[1024x1024],
2.15 GFLOP) on the TensorEngine in fp8-e4m3 with perf_mode=DoubleRow
(the 0/1 connectivity matrix is exact in fp8; x quantization costs
~0.5% final rel err vs the 2e-2 gate).  DoubleRow virtualizes the array
to K=256: 4 chunk-pairs x 8 o-tiles x 2 psum-halves = 64 matmuls of
N=512 at ~242ns warm.

Postprocess per o-tile (PSUM [128 o, (b,d,s)]) is spread so no engine
exceeds the ~2.0us/tile PE pace:
  Scalar: ACT-copy PSUM half(s) -> bf16 (errata: (FD+352)/1.2), tanh,
          sigmoid(+b_soma per-partition bias).
  Vector: w_syn multiplies (bf16 2x_1P after the scalar copy; the
          non-copied half direct from PSUM fp32 at 1x), then the
          s-reduce as a bf16 pairwise tree (2x until the last level),
          then the b_dend bias add.  All in-order on DVE: no cross-engine
          hops inside the chain.
  GpSimd: soma stage (w_dend mult + d-tree), plus mt/params DMA issue.
Even tiles copy both halves (scalar-heavy), odd tiles copy one
(vector-heavy), balancing the two engines at ~1.7us/tile each.

Schedule: o-tiles 0-3 accumulate k-outer over the 4 chunk-pairs, riding
the input DMA stream; tiles 4-7 run k-inner, each tile's postprocess
pipelining against the next tile's matmuls.  Tile 7 runs h-outer so its
first PSUM half is ready ~1us before its last matmul, shortening the
serial tail.  x chunk-pairs load on the Sync HWDGE, mt on GpSimd
(Scalar stays free for postprocess); the first chunk-pair is split in
half so the first real matmul only waits for 2x128KB.
"""

import numpy as np
import ml_dtypes
from contextlib import ExitStack

import concourse.bass as bass
import concourse.mybir as mybir
import concourse.tile as tile

B, N, D, S = 64, 1024, 8, 16
NCORES = 8
BC = B // NCORES          # batches per core = 8
DS = D * S                # 128
P = 128                   # SBUF partitions
KT = N // P               # 8 contraction chunks (input neurons)
KT2 = KT // 2             # 4 DoubleRow chunk pairs (K=256 each)
OT = N // P               # 8 output-neuron tiles
FH = 512                  # matmul moving free dim (one fp32 PSUM bank)
BD = BC * D               # 64
GRP = 4                   # o-tiles in the k-outer leading group

F32 = mybir.dt.float32
BF16 = mybir.dt.bfloat16
FP8 = mybir.dt.float8e4

# packed fp32 params: b_dend | b_soma | w_syn_f32 (all o-tile-major)
PF_BD, PF_BS, PF_WS = 0, OT * D, OT * D + OT
PF_COLS = OT * D + OT + OT * DS          # 1096
# packed bf16 params: w_syn | w_dend
PB_WS, PB_WD = 0, OT * DS
PB_COLS = OT * DS + OT * D               # 1088

_NC_CACHE = {}


def legalize_waits(nc, max_attached=1):
    """Split multi-semaphore waits onto preceding same-engine NOPs.

    The walrus build in this environment accepts at most one sync-wait
    command per instruction (setupSyncWait: "Too many sync wait commands"),
    but Tile attaches one wait per out-of-date engine clock.  An engine is
    in-order, so hoisting the extra waits onto NOPs immediately before the
    instruction is semantics-preserving.
    """
    nid = 0
    for f in nc.m.functions:
        for blk in f.blocks:
            new = []
            changed = False
            for inst in blk.instructions:
                si = inst.sync_info
                if si is not None and si.on_wait and len(si.on_wait) > max_attached:
                    waits = list(si.on_wait)
                    for w in waits[:-max_attached]:
                        nid += 1
                        nop = mybir.InstNoOp(name=f"WSPLIT-{nid}", ins=[], outs=[])
                        nop.engine = inst.engine
                        nop.sync_info = mybir.SyncInfo(on_wait=[w], on_update=[])
                        new.append(nop)
                    inst.sync_info = mybir.SyncInfo(
                        on_wait=waits[-max_attached:], on_update=list(si.on_update)
                    )
                    changed = True
                new.append(inst)
            if changed:
                blk.instructions = new
    return nc


def build_nc(legalize=True):
    """Build the single-core Bass program (SPMD: same program on all cores)."""
    nc = bass.Bass()
    mt = nc.declare_dram_parameter("mt", [N, N], FP8, isOutput=False)
    xc = nc.declare_dram_parameter("xc", [N, BC * DS], FP8, isOutput=False)
    pf32 = nc.declare_dram_parameter("pf32", [P, PF_COLS], F32, isOutput=False)
    pbf = nc.declare_dram_parameter("pbf", [P, PB_COLS], BF16, isOutput=False)
    out = nc.declare_dram_parameter("out", [P, OT * BC], F32, isOutput=True)

    AF = mybir.ActivationFunctionType
    OP = mybir.AluOpType
    DR = mybir.MatmulPerfMode.DoubleRow

    with tile.TileContext(nc) as tc, ExitStack() as ctx:
        wpool = ctx.enter_context(tc.tile_pool(name="weights", bufs=1))
        xpool = ctx.enter_context(tc.tile_pool(name="xin", bufs=1))
        pspool = ctx.enter_context(tc.tile_pool(name="ps", bufs=8, space="PSUM"))
        prpool = ctx.enter_context(tc.tile_pool(name="prp", bufs=3))
        smpool = ctx.enter_context(tc.tile_pool(name="smp", bufs=3))

        # --- PE pre-warm: dummy matmuls on memset scratch while the first
        # input chunk is in flight, lifting the HAM clock gate (1.2 ->
        # 2.4 GHz needs ~3.4us of sustained PE activity). ---
        warm_sb = wpool.tile([P, FH], BF16, tag="warm", name="warm_sb")
        nc.gpsimd.memset(warm_sb[:], 0.0)
        warm_ps = pspool.tile([P, FH], F32, tag="ps", name="warm_ps")
        for _ in range(7):
            nc.tensor.matmul(
                warm_ps[:], lhsT=warm_sb[:, 0:P], rhs=warm_sb[:],
                start=True, stop=True,
            )

        # --- input DMAs: per DoubleRow chunk-pair [128, (g=2, cols)] tiles
        # (contraction row g*128+p).  x on Sync, mt on GpSimd.  The first
        # chunk-pair is split in column halves so the leading group's first
        # matmul waits on only 2x128KB. ---
        x_tiles, mt_tiles = [], []
        x0h, mt0h = [], []
        for h in range(2):
            xt = xpool.tile([P, 2 * FH], FP8, tag=f"x0{h}", name=f"x0{h}")
            nc.sync.dma_start(
                xt[:].rearrange("p (g c) -> p g c", g=2),
                xc[0:2 * P, h * FH:(h + 1) * FH].rearrange(
                    "(g p) c -> p g c", g=2),
            )
            x0h.append(xt)
            mtk = xpool.tile([P, 2 * FH], FP8, tag=f"m0{h}", name=f"m0{h}")
            nc.gpsimd.dma_start(
                mtk[:].rearrange("p (g c) -> p g c", g=2),
                mt[0:2 * P, h * FH:(h + 1) * FH].rearrange(
                    "(g p) c -> p g c", g=2),
            )
            mt0h.append(mtk)
        x_tiles.append(None)
        mt_tiles.append(None)
        for k in range(1, KT2):
            xt = xpool.tile([P, 2 * BC * DS], FP8, tag=f"x{k}", name=f"x{k}")
            mtk = xpool.tile([P, 2 * N], FP8, tag=f"m{k}", name=f"m{k}")
            nc.sync.dma_start(
                xt[:].rearrange("p (g c) -> p g c", g=2),
                xc[k * 2 * P:(k + 1) * 2 * P, :].rearrange(
                    "(g p) c -> p g c", g=2),
            )
            nc.gpsimd.dma_start(
                mtk[:].rearrange("p (g c) -> p g c", g=2),
                mt[k * 2 * P:(k + 1) * 2 * P, :].rearrange(
                    "(g p) c -> p g c", g=2),
            )
            x_tiles.append(xt)
            mt_tiles.append(mtk)

        pbf_sb = wpool.tile([P, PB_COLS], BF16, tag="pbf", name="pbf_sb")
        nc.gpsimd.dma_start(pbf_sb[:], pbf[:, :])
        pf_sb = wpool.tile([P, PF_COLS], F32, tag="pf32", name="pf_sb")
        nc.gpsimd.dma_start(pf_sb[:], pf32[:, :])

        out_sb = wpool.tile([P, OT * BC], F32, tag="out", name="out_sb")

        def wsyn_bf(t):
            return (pbf_sb[:, PB_WS + t * DS:PB_WS + (t + 1) * DS]
                    .unsqueeze(1).broadcast_to([P, BC, DS]))

        def wsyn_f32(t):
            return (pf_sb[:, PF_WS + t * DS:PF_WS + (t + 1) * DS]
                    .unsqueeze(1).broadcast_to([P, BC, DS]))

        def postprocess(t, pst, copy_both):
            # Scalar: PSUM -> SBUF bf16 copies (1, or 2 on scalar-heavy
            # tiles); DVE multiplies by w_syn (bf16 2x after a copy, fp32
            # 1x straight from PSUM otherwise) into one contiguous bf16
            # prod tile, then reduces s as a bf16 pairwise tree and adds
            # b_dend -- the whole DVE chain is same-engine, in-order.
            prod = prpool.tile([P, BC * DS], BF16, tag="prod", name=f"prod{t}")
            nhalf = 2 if copy_both else 1
            for h in range(nhalf):
                cp = prpool.tile([P, FH], BF16, tag=f"cp{h}", name=f"cp{t}_{h}")
                nc.scalar.activation(cp[:], pst[h][:], AF.Copy)
                nc.vector.tensor_mul(
                    prod[:, h * FH:(h + 1) * FH].rearrange(
                        "p (b q) -> p b q", b=BC // 2),
                    cp[:].rearrange("p (b q) -> p b q", b=BC // 2),
                    wsyn_bf(t)[:, h * (BC // 2):(h + 1) * (BC // 2), :],
                )
            if not copy_both:
                nc.vector.tensor_mul(
                    prod[:, FH:2 * FH].rearrange("p (b q) -> p b q", b=BC // 2),
                    pst[1][:].rearrange("p (b q) -> p b q", b=BC // 2),
                    wsyn_f32(t)[:, BC // 2:BC, :],
                )
            # bf16 pairwise tree over s (16 -> 8 -> 4 -> 2 -> 1)
            pv = prod[:].rearrange("p (bd s) -> p bd s", s=S)
            gr1 = smpool.tile([P, BD * 8], BF16, tag="gr1", name=f"gr1{t}")
            g1 = gr1[:].rearrange("p (bd s) -> p bd s", s=8)
            nc.vector.tensor_add(g1, pv[:, :, 0:8], pv[:, :, 8:16])
            gr2 = smpool.tile([P, BD * 4], BF16, tag="gr2", name=f"gr2{t}")
            g2 = gr2[:].rearrange("p (bd s) -> p bd s", s=4)
            nc.vector.tensor_add(g2, g1[:, :, 0:4], g1[:, :, 4:8])
            gr3 = smpool.tile([P, BD * 2], BF16, tag="gr3", name=f"gr3{t}")
            g3 = gr3[:].rearrange("p (bd s) -> p bd s", s=2)
            nc.vector.tensor_add(g3, g2[:, :, 0:2], g2[:, :, 2:4])
            dp = smpool.tile([P, BD], F32, tag="dp", name=f"dp{t}")
            nc.vector.tensor_add(dp[:].unsqueeze(2), g3[:, :, 0:1], g3[:, :, 1:2])
            nc.vector.tensor_add(
                dp[:].rearrange("p (b d) -> p b d", d=D),
                dp[:].rearrange("p (b d) -> p b d", d=D),
                pf_sb[:, PF_BD + t * D:PF_BD + (t + 1) * D].unsqueeze(1)
                .broadcast_to([P, BC, D]),
            )
            dend = smpool.tile([P, BD], BF16, tag="dend", name=f"dend{t}")
            nc.scalar.activation(dend[:], dp[:], AF.Tanh)
            # GpSimd soma: * w_dend then pairwise d-tree (8 -> 4 -> 2 -> 1)
            sp = smpool.tile([P, BD], BF16, tag="sp", name=f"sp{t}")
            spv = sp[:].rearrange("p (b d) -> p b d", d=D)
            nc.gpsimd.tensor_mul(
                spv,
                dend[:].rearrange("p (b d) -> p b d", d=D),
                pbf_sb[:, PB_WD + t * D:PB_WD + (t + 1) * D].unsqueeze(1)
                .broadcast_to([P, BC, D]),
            )
            r1 = smpool.tile([P, BC * 4], BF16, tag="r1", name=f"r1{t}")
            r1v = r1[:].rearrange("p (b d) -> p b d", d=4)
            nc.gpsimd.tensor_add(r1v, spv[:, :, 0:4], spv[:, :, 4:8])
            r2 = smpool.tile([P, BC * 2], BF16, tag="r2", name=f"r2{t}")
            r2v = r2[:].rearrange("p (b d) -> p b d", d=2)
            nc.gpsimd.tensor_add(r2v, r1v[:, :, 0:2], r1v[:, :, 2:4])
            soma = smpool.tile([P, BC], F32, tag="soma", name=f"soma{t}")
            nc.gpsimd.tensor_add(
                soma[:].unsqueeze(2), r2v[:, :, 0:1], r2v[:, :, 1:2])
            nc.scalar.activation(
                out_sb[:, t * BC:(t + 1) * BC], soma[:], AF.Sigmoid,
                bias=pf_sb[:, PF_BS + t:PF_BS + t + 1],
            )

        def mm(pst, t, k, h):
            if k == 0:
                src = x0h[h]
                rhs = src[:].rearrange("p (g c) -> p g c", g=2)[:, :, :]
                msrc = mt0h[t // GRP]
                lhs = (msrc[:].rearrange("p (g c) -> p g c", g=2)
                       [:, :, (t % GRP) * P:(t % GRP + 1) * P])
            else:
                rhs = (x_tiles[k][:].rearrange("p (g c) -> p g c", g=2)
                       [:, :, h * FH:(h + 1) * FH])
                lhs = (mt_tiles[k][:].rearrange("p (g c) -> p g c", g=2)
                       [:, :, t * P:(t + 1) * P])
            nc.tensor.matmul(
                pst[h][:], lhsT=lhs, rhs=rhs,
                start=(k == 0), stop=(k == KT2 - 1), perf_mode=DR,
            )

        # Leading group: k-outer over o-tiles 0..GRP-1 — per-k PE work
        # paces with the chunk-pair DMA stream.
        pst = {}
        for t in range(GRP):
            pst[t] = [
                pspool.tile([P, FH], F32, tag="ps", name=f"ps{t}_{h}")
                for h in range(2)
            ]
        for k in range(KT2):
            for t in range(GRP):
                for h in range(2):
                    mm(pst[t], t, k, h)
        for t in range(GRP):
            postprocess(t, pst[t], copy_both=(t % 2 == 0))

        # Trailing o-tiles: k-inner; each tile's chain overlaps the next
        # tile's matmuls.  The last tile runs h-outer so its first PSUM
        # half completes ~1us before its last matmul.
        for t in range(GRP, OT):
            pstt = [
                pspool.tile([P, FH], F32, tag="ps", name=f"ps{t}_{h}")
                for h in range(2)
            ]
            if t == OT - 1:
                for h in range(2):
                    for k in range(KT2):
                        mm(pstt, t, k, h)
            else:
                for k in range(KT2):
                    for h in range(2):
                        mm(pstt, t, k, h)
            postprocess(t, pstt, copy_both=(t % 2 == 0))

        nc.sync.dma_start(out[:, :], out_sb[:])

    if legalize:
        legalize_waits(nc)
    return nc


def get_nc():
    if "nc" not in _NC_CACHE:
        _NC_CACHE["nc"] = build_nc()
    return _NC_CACHE["nc"]


def pack_params(w_syn, b_dend, w_dend, b_soma):
    """Pack per-neuron parameters into the fp32 and bf16 SBUF layouts
    (each section o-tile-major: column block t holds o-tile t's rows)."""
    ws = np.asarray(w_syn, np.float32).reshape(OT, P, DS).transpose(1, 0, 2).reshape(P, OT * DS)
    bd = np.asarray(b_dend, np.float32).reshape(OT, P, D).transpose(1, 0, 2).reshape(P, OT * D)
    wd = np.asarray(w_dend, np.float32).reshape(OT, P, D).transpose(1, 0, 2).reshape(P, OT * D)
    bs = np.asarray(b_soma, np.float32).reshape(OT, P).T
    pf = np.ascontiguousarray(np.concatenate([bd, bs, ws], axis=1))
    pb = np.ascontiguousarray(
        np.concatenate([ws, wd], axis=1).astype(ml_dtypes.bfloat16))
    return pf, pb


def prepare_in_maps(x, matriz_conexao, w_syn, b_dend, w_dend, b_soma):
    x = np.asarray(x, dtype=np.float32)
    mt_np = np.ascontiguousarray(
        np.asarray(matriz_conexao, np.float32).T).astype(ml_dtypes.float8_e4m3)
    pf, pb = pack_params(w_syn, b_dend, w_dend, b_soma)
    xt = np.ascontiguousarray(x.transpose(1, 0, 2, 3).reshape(N, B, DS))
    in_maps = []
    for c in range(NCORES):
        xc_np = np.ascontiguousarray(
            xt[:, c * BC:(c + 1) * BC, :].reshape(N, BC * DS)
        ).astype(ml_dtypes.float8_e4m3)
        in_maps.append({"mt": mt_np, "xc": xc_np, "pf32": pf, "pbf": pb})
    return in_maps


def assemble_output(results):
    outs = []
    for c in range(NCORES):
        oc = np.asarray(results[c]["out"])          # [P, OT*BC] = (oi, (t, b))
        outs.append(oc.reshape(P, OT, BC).transpose(2, 1, 0).reshape(BC, N))
    return np.ascontiguousarray(np.concatenate(outs, axis=0).astype(np.float32))


def kernel(x, matriz_conexao, w_syn, b_dend, w_dend, b_soma):
    from concourse.bass_utils import run_bass_kernel_spmd
    in_maps = prepare_in_maps(x, matriz_conexao, w_syn, b_dend, w_dend, b_soma)
    nc = get_nc()
    res = run_bass_kernel_spmd(nc, in_maps, list(range(NCORES)))
    return assemble_output(res.results)
